# revision 34
# baseline (speedup 1.0000x reference)
"""AdaptivePatchEmbed Trainium2 kernel.

Distribution: data-parallel over batch B=8 -> one sample per NeuronCore
(descriptors are identical across samples; small conv weights replicated).

Per-core device kernel:
  - scale-0 tokens: 88 static DRAM->DRAM block DMAs (one per 4x4 source
    block; dst rows are contiguous in desc0 order). Falls back to a
    dma_gather path if desc0 lacks the block structure.
  - scale-1 / scale-2 conv inputs: one dma_gather(transpose=True) each,
    which gathers the (i,j)-shifted rows for all tokens and deposits them
    channel-major as [128, D/128, ntok] bf16 -- directly the matmul rhs.
  - convs are einsums tok_out[d, n] = sum_{ij,c} W[(ij,c),d] * X[(ij,c),n],
    run as 128x128-chunk matmuls accumulating in PSUM (K-chunk order
    (i,j,c) matches the host-pretransposed weights).
  - conv2a output is written bf16 channel-major and feeds conv2b in-SBUF.
Outputs per core: out0 [1408,768] f32 (scale-0 copy) and outT [768,420]
f32 (tok1 ++ tok2 transposed). Host reassembles and computes positions.
"""

import numpy as np
import ml_dtypes
from contextlib import ExitStack

# Problem constants (hardcoded; kernel.py must be self-contained).
B, H, W, T, D = 8, 32, 32, 4, 768
N0, N1, N2 = 1408, 336, 84
NPOS = H * W * T              # 4096 rows of D
KC = 24                       # K chunks of 128 over (i,j,c)=4*768
MC = 6                        # output-d chunks of 128
NTOK1 = N1                    # 336 conv1 tokens
NTOK2A = N2 * 4               # 336 conv2a output tokens
N_CORES = 8

_compiled = {}


def _flat_idx(y, x, t):
    return (y * W + x) * T + t


def _wrap_idxs(idx, pad_to):
    """int sequence -> int16 SBUF wrap layout [128, pad_to//16]."""
    idx = np.asarray(idx, np.int64)
    full = np.zeros(pad_to, np.int64)  # pad with valid idx 0 (junk cols, never read)
    full[: idx.size] = idx
    assert full.max() < 32768 and pad_to % 16 == 0
    wrapped = full.reshape(pad_to // 16, 16).T.astype(np.int16)  # [16, cols]
    return np.tile(wrapped, (8, 1))  # replicate across the 8 groups of 16


def _weight_mat(w):
    """w [D, D, 2, 2] -> [128, KC*D] bf16, partition-major so the weight DMA
    moves one fat contiguous run per partition (36.9KB descriptors).
    Logical content: wm[(i*2+j)*D + c, d] = w[d, c, i, j], chunked as
    wm.reshape(KC, 128, D) and laid out [p, kc, d]."""
    wm = np.transpose(np.asarray(w, np.float32), (2, 3, 1, 0))  # [i, j, c, d]
    wm = wm.reshape(4 * D, D).astype(ml_dtypes.bfloat16)
    return np.ascontiguousarray(
        wm.reshape(KC, 128, D).transpose(1, 0, 2)).reshape(128, KC * D)


def _bias_tile(b1, b2a, b2b):
    """[128, 18] f32: cols [g*6+m] = b_g[m*128+p]."""
    out = np.empty((128, 18), np.float32)
    for g, b in enumerate((b1, b2a, b2b)):
        out[:, g * 6 : (g + 1) * 6] = np.asarray(b, np.float32).reshape(MC, 128).T
    return out


def _tok0_blocks(desc0):
    """If desc0 is 88 4x4 raster blocks, return [(by, bx, t), ...] else None."""
    d0 = np.asarray(desc0, np.int64)
    if d0.shape != (N0, 3) or N0 % 16:
        return None
    blocks = d0.reshape(N0 // 16, 16, 3)
    by, bx, t = blocks[:, 0, 0], blocks[:, 0, 1], blocks[:, 0, 2]
    yy = by[:, None] + np.repeat(np.arange(4), 4)[None, :]
    xx = bx[:, None] + np.tile(np.arange(4), 4)[None, :]
    tt = np.broadcast_to(t[:, None], yy.shape)
    exp = np.stack([yy, xx, tt], axis=2)
    if not np.array_equal(exp, blocks) or yy.max() >= H or xx.max() >= W:
        return None
    return list(zip(by.tolist(), bx.tolist(), t.tolist()))


def _tok0_runs(tok0_blocks):
    """Batch the scale-0 copy into few fat DMAs.

    Requires the spatial 4x4 block set to be identical across all T
    timesteps and 4-aligned. Returns (runs, perm):
      runs: [(by, bx_blk0, step_blk, L)] -- each run is one DRAM->DRAM DMA
            covering [4 y, L blocks strided, 16 rows (4x * 4t)] of base.
      perm: int array s.t. out0_final = out0_raw[perm] (host-side reorder).
    """
    if tok0_blocks is None:
        return None, None
    byt = {}
    for by, bx, t in tok0_blocks:
        if by % 4 or bx % 4:
            return None, None
        byt.setdefault((by, bx), set()).add(t)
    if any(ts != set(range(T)) for ts in byt.values()):
        return None, None
    if len(byt) * 16 * T != N0:
        return None, None
    rows = {}
    for (by, bx) in byt:
        rows.setdefault(by, []).append(bx // 4)
    runs = []
    for by in sorted(rows):
        bxs = sorted(rows[by])
        i = 0
        while i < len(bxs):
            j = i + 1
            step = 1
            if j < len(bxs):
                step = bxs[j] - bxs[i]
                while j + 1 < len(bxs) and bxs[j + 1] - bxs[j] == step:
                    j += 1
            runs.append((by, bxs[i], step, j - i))
            i = j
    # raw row order produced by the DMAs
    raw = []
    for by, b0, s, L in runs:
        for dy in range(4):
            for k in range(L):
                for dx in range(4):
                    for t in range(T):
                        raw.append(((by + dy) * W + (b0 + k * s) * 4 + dx) * T + t)
    raw = np.asarray(raw)
    pos_of = np.full(NPOS, -1, np.int64)
    pos_of[raw] = np.arange(N0)
    return runs, pos_of


def _build_bass(tok0_runs):
    import concourse.bacc as bacc
    import concourse.tile as tile
    from concourse import mybir

    nc = bacc.Bacc("TRN2", target_bir_lowering=False, debug=False,
                   num_devices=N_CORES, num_swdge_queues=1,
                   dynamic_dma_scratch_size=32768)
    dt = mybir.dt

    base_f32 = nc.dram_tensor("base_f32", (NPOS, D), dt.float32, kind="ExternalInput")
    base_bf16 = nc.dram_tensor("base_bf16", (NPOS, D), dt.bfloat16, kind="ExternalInput")
    w1m = nc.dram_tensor("w1m", (128, KC * D), dt.bfloat16, kind="ExternalInput")
    w2am = nc.dram_tensor("w2am", (128, KC * D), dt.bfloat16, kind="ExternalInput")
    w2bm = nc.dram_tensor("w2bm", (128, KC * D), dt.bfloat16, kind="ExternalInput")
    biases = nc.dram_tensor("biases", (128, 18), dt.float32, kind="ExternalInput")
    n_idx_cols = 192 if tok0_runs is not None else 280
    idxs = nc.dram_tensor("idxs", (128, n_idx_cols), dt.int16, kind="ExternalInput")
    out0 = nc.dram_tensor("out0", (N0, D), dt.float32, kind="ExternalOutput")
    outT = nc.dram_tensor("outT", (D, NTOK1 + N2), dt.float32, kind="ExternalOutput")

    with ExitStack() as ctx:
        tc = ctx.enter_context(tile.TileContext(nc))
        consts = ctx.enter_context(tc.tile_pool(name="consts", bufs=1))
        wpool = ctx.enter_context(tc.tile_pool(name="wpool", bufs=1))
        gpool = ctx.enter_context(tc.tile_pool(name="gpool", bufs=1))
        opool = ctx.enter_context(tc.tile_pool(name="opool", bufs=1))
        psum = ctx.enter_context(tc.tile_pool(name="psum", bufs=4, space="PSUM"))

        # idx upload first on the sync ring: it gates the gathers (critical path)
        idx_s = consts.tile([128, n_idx_cols], dt.int16)
        nc.sync.dma_start(idx_s[:], idxs.ap()[:])
        bias_s = consts.tile([128, 18], dt.float32)
        nc.scalar.dma_start(bias_s[:], biases.ap()[:])

        # conv gathers, split in halves (2 ij-groups each) so matmuls can
        # chase the gather data. The gpsimd dma_gather library load (~13-18us
        # after the preamble) is the hard floor before the first one runs.
        ghalves = []
        for gi in range(4):
            gh = gpool.tile([128, MC, 768], dt.bfloat16, tag=f"gh{gi}")
            nc.gpsimd.dma_gather(
                gh[:], base_bf16.ap()[:], idx_s[:, gi * 48 : gi * 48 + 48],
                num_idxs=768, num_idxs_reg=768, elem_size=D, transpose=True,
                single_packet=False,
            )
            ghalves.append(gh)

        def rhs_conv(gi_base, ij, c6, ntok):
            half = ghalves[gi_base + ij // 2]
            ijl = ij % 2
            return half[:, c6, ijl * ntok : (ijl + 1) * ntok]

        # Weights -> SBUF [128, KC, D]; DRAM layout is partition-major so
        # descriptors are fat contiguous runs, but capped at 4.6KB so the
        # SDMA packet round-robin does not starve the concurrent gathers.
        wts = []
        for wdram, nm in ((w1m, "w1"), (w2am, "w2a"), (w2bm, "w2b")):
            wt = wpool.tile([128, KC, D], dt.bfloat16, tag=nm)
            nc.sync.dma_start(wt[:], wdram.ap().rearrange("p (k d) -> p k d", d=D),
                              max_dma_last_dim=2304)
            wts.append(wt)
        w1s, w2as, w2bs = wts

        # conv1: out1T[d, n] over 336 tokens
        out1 = opool.tile([128, MC, NTOK1], dt.float32, tag="out1")
        for m in range(MC):
            ps = psum.tile([128, NTOK1], dt.float32, tag="ps")
            for kc in range(KC):
                ij, c6 = divmod(kc, MC)
                nc.tensor.matmul(
                    ps[:],
                    w1s[:, kc, m * 128 : (m + 1) * 128],
                    rhs_conv(0, ij, c6, NTOK1),
                    start=(kc == 0), stop=(kc == KC - 1),
                )
            nc.vector.tensor_scalar_add(out1[:, m, :], ps[:], bias_s[:, m : m + 1])

        # conv2a: 336 output tokens; token order inside each (i,j) group is
        # (h, w, n) so conv2b rhs slices are contiguous
        out2a = opool.tile([128, MC, NTOK2A], dt.bfloat16, tag="out2a")
        for m in range(MC):
            ps = psum.tile([128, NTOK2A], dt.float32, tag="ps")
            for kc in range(KC):
                ij, c6 = divmod(kc, MC)
                nc.tensor.matmul(
                    ps[:],
                    w2as[:, kc, m * 128 : (m + 1) * 128],
                    rhs_conv(2, ij, c6, NTOK2A),
                    start=(kc == 0), stop=(kc == KC - 1),
                )
            nc.vector.tensor_scalar_add(out2a[:, m, :], ps[:], bias_s[:, 6 + m : 7 + m])

        # conv2b: contracts conv2a output; group (i,j) -> cols [(2i+j)*84, +84)
        out2b = opool.tile([128, MC, N2], dt.float32, tag="out2b")
        for m in range(MC):
            ps = psum.tile([128, N2], dt.float32, tag="ps")
            for kc in range(KC):
                ij, c6 = divmod(kc, MC)
                nc.tensor.matmul(
                    ps[:],
                    w2bs[:, kc, m * 128 : (m + 1) * 128],
                    out2a[:, c6, ij * N2 : (ij + 1) * N2],
                    start=(kc == 0), stop=(kc == KC - 1),
                )
            nc.vector.tensor_scalar_add(out2b[:, m, :], ps[:], bias_s[:, 12 + m : 13 + m])

        # scale-0: few fat static DRAM->DRAM copies in SOURCE order (the
        # host applies the row permutation when assembling the output).
        # On the scalar ring, which is otherwise idle.
        if tok0_runs is not None:
            base_y = base_f32.ap().rearrange("(y bx rr) d -> y bx rr d",
                                             bx=W // 4, rr=4 * T)
            off = 0
            for by, b0, s, L in tok0_runs:
                nrows = 4 * L * 4 * T
                nc.scalar.dma_start(
                    out0.ap()[off : off + nrows, :],
                    base_y[by : by + 4, b0 : b0 + (L - 1) * s + 1 : s, :, :],
                )
                off += nrows
        else:
            g0 = gpool.tile([128, N0 // 128, D], dt.float32, tag="g0")
            nc.gpsimd.dma_gather(
                g0[:], base_f32.ap()[:], idx_s[:, 192:280],
                num_idxs=N0, num_idxs_reg=N0, elem_size=D, single_packet=False,
            )
            nc.sync.dma_start(
                out0.ap().rearrange("(g p) d -> p g d", p=128), g0[:]
            )

        # outputs: outT [768, 420] viewed [6, 128, 420]
        outT_v = outT.ap().rearrange("(m p) n -> p m n", p=128)
        nc.sync.dma_start(outT_v[:, :, 0:NTOK1], out1[:])
        nc.sync.dma_start(outT_v[:, :, NTOK1 : NTOK1 + N2], out2b[:])

    nc.finalize()
    return nc


def _prep_shared(desc0, desc1, desc2, w1, b1, w2a, b2a, w2b, b2b):
    """Host-side shared (core-independent) input prep."""
    d0 = np.asarray(desc0, np.int64)
    d1 = np.asarray(desc1, np.int64)
    d2 = np.asarray(desc2, np.int64)

    tok0_blocks = _tok0_blocks(d0)
    tok0_runs, pos_of = _tok0_runs(tok0_blocks)
    if tok0_runs is not None:
        d0flat = _flat_idx(d0[:, 0], d0[:, 1], d0[:, 2])
        perm = pos_of[d0flat]
        assert perm.min() >= 0
    else:
        perm = None

    # conv1: (i,j)-major groups, desc1 order inside
    idx1_groups = [
        _flat_idx(d1[:, 0] + i, d1[:, 1] + j, d1[:, 2])
        for i in range(2) for j in range(2)
    ]                                                                 # 4 x [336]

    # conv2a: (i,j)-major groups; token order inside = (h, w, n)
    hh, ww = np.arange(2), np.arange(2)
    idx2_groups = [
        _flat_idx(
            (d2[:, 0][None, None, :] + 2 * hh[:, None, None] + i),
            (d2[:, 1][None, None, :] + 2 * ww[None, :, None] + j),
            np.broadcast_to(d2[:, 2][None, None, :], (2, 2, N2)),
        ).ravel()
        for i in range(2) for j in range(2)
    ]                                                                 # 4 x [336]

    halves = [np.concatenate(idx1_groups[0:2]), np.concatenate(idx1_groups[2:4]),
              np.concatenate(idx2_groups[0:2]), np.concatenate(idx2_groups[2:4])]
    parts = [_wrap_idxs(h, 768) for h in halves]
    if tok0_runs is None:
        idx0 = _flat_idx(d0[:, 0], d0[:, 1], d0[:, 2])                # [1408]
        parts.append(_wrap_idxs(idx0, 1408))
    idxs = np.concatenate(parts, axis=1)

    shared = {
        "w1m": _weight_mat(w1),
        "w2am": _weight_mat(w2a),
        "w2bm": _weight_mat(w2b),
        "biases": _bias_tile(b1, b2a, b2b),
        "idxs": idxs,
    }

    def _pos(desc, size):
        n = desc.shape[0]
        return np.concatenate(
            [desc[:, :2].astype(np.int32),
             np.full((n, 1), size, np.int32),
             desc[:, 2:3].astype(np.int32)], axis=1)

    positions = np.concatenate(
        [_pos(np.asarray(desc0, np.int32), 1),
         _pos(np.asarray(desc1, np.int32), 2),
         _pos(np.asarray(desc2, np.int32), 4)], axis=0)
    return shared, positions, tok0_runs, perm


def kernel(base_patch_embeddings, desc0, desc1, desc2,
           w1, b1, w2a, b2a, w2b, b2b):
    from concourse.bass_utils import run_bass_kernel_spmd

    base = np.asarray(base_patch_embeddings, np.float32)
    assert base.shape == (B, H, W, T, D)

    shared, positions, tok0_runs, perm = _prep_shared(
        desc0, desc1, desc2, w1, b1, w2a, b2a, w2b, b2b)

    key = repr(tok0_runs)
    if key not in _compiled:
        _compiled[key] = _build_bass(tok0_runs)
    nc = _compiled[key]

    in_maps = []
    for b in range(B):
        sample = np.ascontiguousarray(base[b].reshape(NPOS, D))
        m = dict(shared)
        m["base_f32"] = sample
        m["base_bf16"] = sample.astype(ml_dtypes.bfloat16)
        in_maps.append(m)

    res = run_bass_kernel_spmd(nc, in_maps, core_ids=list(range(N_CORES)))

    tokens = np.empty((B, N0 + N1 + N2, D), np.float32)
    for b in range(B):
        out0 = res.results[b]["out0"]
        tokens[b, :N0] = out0[perm] if perm is not None else out0
        tokens[b, N0:] = res.results[b]["outT"].T
    return tokens, positions


# revision 37
# speedup vs baseline: 1.1451x; 1.1451x over previous
"""AdaptivePatchEmbed Trainium2 kernel.

Distribution: data-parallel over batch B=8 -> one sample per NeuronCore
(descriptors are identical per sample; small conv weights replicated).

Per-core device kernel (fast path, used when the descriptors have the
block structure that _build_descs produces; a generic dma_gather path
is the fallback):
  - scale-0 tokens: ~8 fat static DRAM->DRAM copies in source order (the
    host applies the row permutation when assembling the output).
  - conv inputs: one dma_gather(transpose=True) per conv with 12.3KB
    "super-row" elements (2 x-columns x 4 timesteps x 768 ch, contiguous
    in DRAM) so only 256 indices per conv are needed; the gather deposits
    them channel-major as [128, 48, 256] bf16, which the matmuls read
    directly with strided access patterns.
  - convs are einsums tok_out[d, n] = sum_{ij,c} W[(ij,c),d] * X[(ij,c),n],
    run as 128x128-chunk matmuls accumulating in PSUM (K-chunk order
    (i,j,c) matches the host-pretransposed weights).
  - conv2a output is written bf16 channel-major and feeds conv2b in-SBUF.
Outputs per core: out0 [1408,768] f32 (scale-0 rows) and outT [768,420]
f32 (tok1 ++ tok2 transposed). Host reassembles and computes positions.
"""

import numpy as np
import ml_dtypes
from contextlib import ExitStack

# Problem constants (hardcoded; kernel.py must be self-contained).
B, H, W, T, D = 8, 32, 32, 4, 768
N0, N1, N2 = 1408, 336, 84
NPOS = H * W * T              # 4096 rows of D
KC = 24                       # K chunks of 128 over (i,j,c)=4*768
MC = 6                        # output-d chunks of 128
NTOK1 = N1                    # 336 conv1 tokens
NTOK2A = N2 * 4               # 336 conv2a output tokens
NSP = 84                      # spatial tokens per timestep (both convs)
N_CORES = 8

_compiled = {}


def _flat_idx(y, x, t):
    return (y * W + x) * T + t


def _wrap_idxs(idx, pad_to):
    """int sequence -> int16 SBUF wrap layout [128, pad_to//16]."""
    idx = np.asarray(idx, np.int64)
    full = np.zeros(pad_to, np.int64)  # pad with valid idx 0 (junk cols, never read)
    full[: idx.size] = idx
    assert full.max() < 32768 and pad_to % 16 == 0
    wrapped = full.reshape(pad_to // 16, 16).T.astype(np.int16)  # [16, cols]
    return np.tile(wrapped, (8, 1))  # replicate across the 8 groups of 16


def _weight_mat(w):
    """w [D, D, 2, 2] -> [128, KC*D] bf16, partition-major so the weight DMA
    moves one contiguous run per partition. Logical content:
    wm[(i*2+j)*D + c, d] = w[d, c, i, j], chunked [KC, 128, D] -> [p, kc, d]."""
    wm = np.transpose(np.asarray(w, np.float32), (2, 3, 1, 0))  # [i, j, c, d]
    wm = wm.reshape(4 * D, D).astype(ml_dtypes.bfloat16)
    return np.ascontiguousarray(
        wm.reshape(KC, 128, D).transpose(1, 0, 2)).reshape(128, KC * D)


def _bias_tile(b1, b2a, b2b):
    """[128, 18] f32: cols [g*6+m] = b_g[m*128+p]."""
    out = np.empty((128, 18), np.float32)
    for g, b in enumerate((b1, b2a, b2b)):
        out[:, g * 6 : (g + 1) * 6] = np.asarray(b, np.float32).reshape(MC, 128).T
    return out


def _tok0_blocks(desc0):
    """If desc0 is 88 4x4 raster blocks, return [(by, bx, t), ...] else None."""
    d0 = np.asarray(desc0, np.int64)
    if d0.shape != (N0, 3) or N0 % 16:
        return None
    blocks = d0.reshape(N0 // 16, 16, 3)
    by, bx, t = blocks[:, 0, 0], blocks[:, 0, 1], blocks[:, 0, 2]
    yy = by[:, None] + np.repeat(np.arange(4), 4)[None, :]
    xx = bx[:, None] + np.tile(np.arange(4), 4)[None, :]
    tt = np.broadcast_to(t[:, None], yy.shape)
    exp = np.stack([yy, xx, tt], axis=2)
    if not np.array_equal(exp, blocks) or yy.max() >= H or xx.max() >= W:
        return None
    return list(zip(by.tolist(), bx.tolist(), t.tolist()))


def _tok0_runs(tok0_blocks):
    """Batch the scale-0 copy into few fat DMAs (see module docstring).
    Returns (runs, pos_of) or (None, None)."""
    if tok0_blocks is None:
        return None, None
    byt = {}
    for by, bx, t in tok0_blocks:
        if by % 4 or bx % 4:
            return None, None
        byt.setdefault((by, bx), set()).add(t)
    if any(ts != set(range(T)) for ts in byt.values()):
        return None, None
    if len(byt) * 16 * T != N0:
        return None, None
    rows = {}
    for (by, bx) in byt:
        rows.setdefault(by, []).append(bx // 4)
    runs = []
    for by in sorted(rows):
        bxs = sorted(rows[by])
        i = 0
        while i < len(bxs):
            j = i + 1
            step = 1
            if j < len(bxs):
                step = bxs[j] - bxs[i]
                while j + 1 < len(bxs) and bxs[j + 1] - bxs[j] == step:
                    j += 1
            runs.append((by, bxs[i], step, j - i))
            i = j
    raw = []
    for by, b0, s, L in runs:
        for dy in range(4):
            for k in range(L):
                for dx in range(4):
                    for t in range(T):
                        raw.append(((by + dy) * W + (b0 + k * s) * 4 + dx) * T + t)
    raw = np.asarray(raw)
    pos_of = np.full(NPOS, -1, np.int64)
    pos_of[raw] = np.arange(N0)
    return runs, pos_of


def _spatial_lists(d1, d2):
    """Per-timestep spatial token lists, or None if structure doesn't hold.

    Requires desc1/desc2 to be t-major with an identical spatial pattern per
    timestep, even coordinates, and in-bounds 2x2 / 4x4 windows. Returns
    (s1_yx [84,2], s2_yx [21,2])."""
    d1 = np.asarray(d1, np.int64)
    d2 = np.asarray(d2, np.int64)
    if d1.shape != (N1, 3) or d2.shape != (N2, 3) or N1 % T or N2 % T:
        return None
    s1 = d1.reshape(T, N1 // T, 3)
    s2 = d2.reshape(T, N2 // T, 3)
    for s, win in ((s1, 2), (s2, 4)):
        if not np.array_equal(s[:, :, 2], np.broadcast_to(
                np.arange(T)[:, None], s.shape[:2])):
            return None
        if not all(np.array_equal(s[0, :, :2], s[t, :, :2]) for t in range(T)):
            return None
        yx = s[0, :, :2]
        if (yx % 2).any() or yx[:, 0].max() + win > H or yx[:, 1].max() + win > W:
            return None
    return s1[0, :, :2], s2[0, :, :2]


def _super_idxs(s1_yx, s2_yx):
    """Super-row gather indices (base viewed as [512, 8*768]): one element
    covers (y, x..x+1, all t). Returns (idx1 [168], idx2 [168])."""
    def srow(y, x):
        return y * (W // 2) + x // 2

    idx1 = np.concatenate([
        np.asarray([srow(y + i, x) for y, x in s1_yx]) for i in range(2)])
    # conv2a spatial order (blk, h, w) inside each i-group
    idx2 = np.concatenate([
        np.asarray([srow(by + 2 * h + i, bx + 2 * w)
                    for by, bx in s2_yx for h in range(2) for w in range(2)])
        for i in range(2)])
    return idx1, idx2


def _build_bass(tok0_runs, fast):
    import concourse.bacc as bacc
    import concourse.tile as tile
    from concourse import mybir

    nc = bacc.Bacc("TRN2", target_bir_lowering=False, debug=False,
                   num_devices=N_CORES, num_swdge_queues=1,
                   dynamic_dma_scratch_size=32768)
    dt = mybir.dt

    base_f32 = nc.dram_tensor("base_f32", (NPOS, D), dt.float32, kind="ExternalInput")
    base_bf16 = nc.dram_tensor("base_bf16", (NPOS, D), dt.bfloat16, kind="ExternalInput")
    w1m = nc.dram_tensor("w1m", (128, KC * D), dt.bfloat16, kind="ExternalInput")
    w2am = nc.dram_tensor("w2am", (128, KC * D), dt.bfloat16, kind="ExternalInput")
    w2bm = nc.dram_tensor("w2bm", (128, KC * D), dt.bfloat16, kind="ExternalInput")
    biases = nc.dram_tensor("biases", (128, 18), dt.float32, kind="ExternalInput")
    n_idx_cols = 32 if fast else 280
    idxs = nc.dram_tensor("idxs", (128, n_idx_cols), dt.int16, kind="ExternalInput")
    out0 = nc.dram_tensor("out0", (N0, D), dt.float32, kind="ExternalOutput")
    outT = nc.dram_tensor("outT", (D, NTOK1 + N2), dt.float32, kind="ExternalOutput")

    with ExitStack() as ctx:
        tc = ctx.enter_context(tile.TileContext(nc))
        consts = ctx.enter_context(tc.tile_pool(name="consts", bufs=1))
        wpool = ctx.enter_context(tc.tile_pool(name="wpool", bufs=1))
        gpool = ctx.enter_context(tc.tile_pool(name="gpool", bufs=1))
        opool = ctx.enter_context(tc.tile_pool(name="opool", bufs=1))
        psum = ctx.enter_context(tc.tile_pool(name="psum", bufs=4, space="PSUM"))

        # idx upload first on the sync ring: it gates the gathers
        idx_s = consts.tile([128, n_idx_cols], dt.int16)
        nc.sync.dma_start(idx_s[:], idxs.ap()[:])
        bias_s = consts.tile([128, 18], dt.float32)
        nc.scalar.dma_start(bias_s[:], biases.ap()[:])

        if fast:
            # Super-row gathers: 256 idxs of 12.3KB elements each; output
            # [128, 48, 256] with middle dim e = (j*4 + t)*6 + c6.
            in_v = base_bf16.ap().rearrange("(s a) d -> s (a d)", a=8)
            g1s = gpool.tile([128, 48, 256], dt.bfloat16, tag="g1")
            nc.gpsimd.dma_gather(
                g1s[:], in_v, idx_s[:, 0:16],
                num_idxs=256, num_idxs_reg=256, elem_size=8 * D, transpose=True,
                single_packet=False,
            )
            g2s = gpool.tile([128, 48, 256], dt.bfloat16, tag="g2a")
            nc.gpsimd.dma_gather(
                g2s[:], in_v, idx_s[:, 16:32],
                num_idxs=256, num_idxs_reg=256, elem_size=8 * D, transpose=True,
                single_packet=False,
            )

            def rhs_conv(g, ij, c6, _ntok):
                i, j = divmod(ij, 2)
                # cols (t, n): e = j*24 + c6 + 6*t ; n-group at i*84
                return g[:, j * 24 + c6 : j * 24 + c6 + 19 : 6,
                         i * NSP : (i + 1) * NSP]
        else:
            # generic fallback: per-row transpose gathers in halves
            ghalves = []
            for gi in range(4):
                gh = gpool.tile([128, MC, 768], dt.bfloat16, tag=f"gh{gi}")
                nc.gpsimd.dma_gather(
                    gh[:], base_bf16.ap()[:], idx_s[:, gi * 48 : gi * 48 + 48],
                    num_idxs=768, num_idxs_reg=768, elem_size=D, transpose=True,
                    single_packet=False,
                )
                ghalves.append(gh)

            def rhs_conv(gi_base, ij, c6, ntok):
                half = ghalves[gi_base + ij // 2]
                ijl = ij % 2
                return half[:, c6, ijl * ntok : (ijl + 1) * ntok]

        g1_key = g1s if fast else 0
        g2_key = g2s if fast else 2

        # Weights -> SBUF [128, KC, D]; DRAM layout partition-major.
        # w1 loads fat (before the gather window); w2a streams with small
        # descriptors during the gather window (big HWDGE descriptors starve
        # SWDGE gather data in the SDMA packet round-robin); w2b fat again.
        wts = []
        for wdram, nm, cap in ((w1m, "w1", None), (w2am, "w2a", 768),
                               (w2bm, "w2b", None)):
            wt = wpool.tile([128, KC, D], dt.bfloat16, tag=nm)
            nc.sync.dma_start(wt[:], wdram.ap().rearrange("p (k d) -> p k d", d=D),
                              max_dma_last_dim=cap)
            wts.append(wt)
        w1s, w2as, w2bs = wts

        # conv1: out1T[d, n] over 336 tokens (col order (t, n) in fast mode)
        out1 = opool.tile([128, MC, NTOK1], dt.float32, tag="out1")
        for m in range(MC):
            ps = psum.tile([128, NTOK1], dt.float32, tag="ps")
            for kc in range(KC):
                ij, c6 = divmod(kc, MC)
                nc.tensor.matmul(
                    ps[:],
                    w1s[:, kc, m * 128 : (m + 1) * 128],
                    rhs_conv(g1_key, ij, c6, NTOK1),
                    start=(kc == 0), stop=(kc == KC - 1),
                )
            nc.vector.tensor_scalar_add(out1[:, m, :], ps[:], bias_s[:, m : m + 1])

        # conv2a: 336 output tokens; fast mode col order (t, blk, h, w),
        # generic col order (ij-group)(h, w, n)
        out2a = opool.tile([128, MC, NTOK2A], dt.bfloat16, tag="out2a")
        for m in range(MC):
            ps = psum.tile([128, NTOK2A], dt.float32, tag="ps")
            for kc in range(KC):
                ij, c6 = divmod(kc, MC)
                nc.tensor.matmul(
                    ps[:],
                    w2as[:, kc, m * 128 : (m + 1) * 128],
                    rhs_conv(g2_key, ij, c6, NTOK2A),
                    start=(kc == 0), stop=(kc == KC - 1),
                )
            nc.vector.tensor_scalar_add(out2a[:, m, :], ps[:], bias_s[:, 6 + m : 7 + m])

        # conv2b contracts conv2a output over its 2x2 spatial positions
        out2b = opool.tile([128, MC, N2], dt.float32, tag="out2b")
        for m in range(MC):
            ps = psum.tile([128, N2], dt.float32, tag="ps")
            for kc in range(KC):
                ij, c6 = divmod(kc, MC)
                if fast:
                    # cols (t*21 + blk)*4 + (2i+j) -> stride-4 slice
                    rhs2b = out2a[:, c6, ij : ij + 4 * (N2 - 1) + 1 : 4]
                else:
                    rhs2b = out2a[:, c6, ij * N2 : (ij + 1) * N2]
                nc.tensor.matmul(
                    ps[:],
                    w2bs[:, kc, m * 128 : (m + 1) * 128],
                    rhs2b,
                    start=(kc == 0), stop=(kc == KC - 1),
                )
            nc.vector.tensor_scalar_add(out2b[:, m, :], ps[:], bias_s[:, 12 + m : 13 + m])

        # scale-0: fat static DRAM->DRAM copies in source order, scalar ring
        if tok0_runs is not None:
            base_y = base_f32.ap().rearrange("(y bx rr) d -> y bx rr d",
                                             bx=W // 4, rr=4 * T)
            off = 0
            for by, b0, s, L in tok0_runs:
                nrows = 4 * L * 4 * T
                nc.scalar.dma_start(
                    out0.ap()[off : off + nrows, :],
                    base_y[by : by + 4, b0 : b0 + (L - 1) * s + 1 : s, :, :],
                )
                off += nrows
        else:
            g0 = gpool.tile([128, N0 // 128, D], dt.float32, tag="g0")
            nc.gpsimd.dma_gather(
                g0[:], base_f32.ap()[:], idx_s[:, 192:280],
                num_idxs=N0, num_idxs_reg=N0, elem_size=D, single_packet=False,
            )
            nc.sync.dma_start(
                out0.ap().rearrange("(g p) d -> p g d", p=128), g0[:]
            )

        # outputs: outT [768, 420] viewed [6, 128, 420]
        outT_v = outT.ap().rearrange("(m p) n -> p m n", p=128)
        nc.sync.dma_start(outT_v[:, :, 0:NTOK1], out1[:])
        nc.sync.dma_start(outT_v[:, :, NTOK1 : NTOK1 + N2], out2b[:])

    nc.finalize()
    return nc


def _prep_shared(desc0, desc1, desc2, w1, b1, w2a, b2a, w2b, b2b):
    """Host-side shared (core-independent) input prep."""
    d0 = np.asarray(desc0, np.int64)
    d1 = np.asarray(desc1, np.int64)
    d2 = np.asarray(desc2, np.int64)

    tok0_blocks = _tok0_blocks(d0)
    tok0_runs, pos_of = _tok0_runs(tok0_blocks)
    if tok0_runs is not None:
        d0flat = _flat_idx(d0[:, 0], d0[:, 1], d0[:, 2])
        perm = pos_of[d0flat]
        assert perm.min() >= 0
    else:
        perm = None

    sp = _spatial_lists(d1, d2)
    fast = sp is not None and tok0_runs is not None

    if fast:
        idx1, idx2 = _super_idxs(*sp)
        idxs = np.concatenate(
            [_wrap_idxs(idx1, 256), _wrap_idxs(idx2, 256)], axis=1)
    else:
        tok0_runs, perm = None, None
        # generic: per-row gathers, (i,j)-major groups
        idx1_groups = [
            _flat_idx(d1[:, 0] + i, d1[:, 1] + j, d1[:, 2])
            for i in range(2) for j in range(2)
        ]
        hh, ww = np.arange(2), np.arange(2)
        idx2_groups = [
            _flat_idx(
                (d2[:, 0][None, None, :] + 2 * hh[:, None, None] + i),
                (d2[:, 1][None, None, :] + 2 * ww[None, :, None] + j),
                np.broadcast_to(d2[:, 2][None, None, :], (2, 2, N2)),
            ).ravel()
            for i in range(2) for j in range(2)
        ]
        halves = [np.concatenate(idx1_groups[0:2]), np.concatenate(idx1_groups[2:4]),
                  np.concatenate(idx2_groups[0:2]), np.concatenate(idx2_groups[2:4])]
        parts = [_wrap_idxs(h, 768) for h in halves]
        idx0 = _flat_idx(d0[:, 0], d0[:, 1], d0[:, 2])
        parts.append(_wrap_idxs(idx0, 1408))
        idxs = np.concatenate(parts, axis=1)

    shared = {
        "w1m": _weight_mat(w1),
        "w2am": _weight_mat(w2a),
        "w2bm": _weight_mat(w2b),
        "biases": _bias_tile(b1, b2a, b2b),
        "idxs": idxs,
    }

    def _pos(desc, size):
        n = desc.shape[0]
        return np.concatenate(
            [desc[:, :2].astype(np.int32),
             np.full((n, 1), size, np.int32),
             desc[:, 2:3].astype(np.int32)], axis=1)

    positions = np.concatenate(
        [_pos(np.asarray(desc0, np.int32), 1),
         _pos(np.asarray(desc1, np.int32), 2),
         _pos(np.asarray(desc2, np.int32), 4)], axis=0)
    return shared, positions, tok0_runs, perm, fast


def kernel(base_patch_embeddings, desc0, desc1, desc2,
           w1, b1, w2a, b2a, w2b, b2b):
    from concourse.bass_utils import run_bass_kernel_spmd

    base = np.asarray(base_patch_embeddings, np.float32)
    assert base.shape == (B, H, W, T, D)

    shared, positions, tok0_runs, perm, fast = _prep_shared(
        desc0, desc1, desc2, w1, b1, w2a, b2a, w2b, b2b)

    key = (repr(tok0_runs), fast)
    if key not in _compiled:
        _compiled[key] = _build_bass(tok0_runs, fast)
    nc = _compiled[key]

    in_maps = []
    for b in range(B):
        sample = np.ascontiguousarray(base[b].reshape(NPOS, D))
        m = dict(shared)
        m["base_f32"] = sample
        m["base_bf16"] = sample.astype(ml_dtypes.bfloat16)
        in_maps.append(m)

    res = run_bass_kernel_spmd(nc, in_maps, core_ids=list(range(N_CORES)))

    tokens = np.empty((B, N0 + N1 + N2, D), np.float32)
    for b in range(B):
        out0 = res.results[b]["out0"]
        tokens[b, :N0] = out0[perm] if perm is not None else out0
        tokens[b, N0:] = res.results[b]["outT"].T
    return tokens, positions
